# revision 10
# baseline (speedup 1.0000x reference)
"""Causal self-attention (separate heads) TRN2 Bass kernel — bf16 rewrite.

Problem (hardcoded): B=4, T=2048, C=1024, H=16, HS=64, fp32 in/out.
  q/k/v = per-head linear projections of x; att = softmax(causal(q k^T / 8));
  y = att v; out = concat_heads(y) @ Wp.T + bp.

Sharding over 8 NeuronCores: core c -> batch b = c//2, head-group hg = c%2
(8 heads each = 4 pairs of heads). Each core computes a [T, C] partial of the
output (its heads' contribution through the column slice of Wp); host sums the
two partials per batch and adds bp.

v2 design (vs fp32r baseline):
  - all matmul operands bf16 (1 col/cycle at any free size, FWL weight loads,
    half DMA + SBUF footprint); PSUM stays fp32.
  - x^T and all weights SBUF-resident, loaded once up front.
  - single PE emission chain; V projection and next-pair QK projection and the
    C phase are interleaved into the attention stages as PE filler so the PE
    never idles (HAM stays at K=8/8 = 2.4 GHz).
  - softmax tail: denominator row from ones-row-augmented V; reciprocal via
    the fast custom-DVE op (~5x cheaper than nc.vector.reciprocal), gpsimd
    partition broadcast, one DVE mul into bf16 YT.
  - PSUM: 4-slot shared pool (S blocks + all projection groups) + 2x2 banks
    for the psY accumulators = exactly 8 banks.
"""
import ml_dtypes
import numpy as np

from concourse import bacc, bass_utils, tile, mybir

B, T, C, H, HS = 4, 2048, 1024, 16, 64
NCORE = 8
NPAIR = 4
NCH = T // 512  # 4 t-chunks of 512
NST = T // 128  # 16 s/t-blocks of 128

f32 = mybir.dt.float32
bf16 = mybir.dt.bfloat16
EXP = mybir.ActivationFunctionType.Exp
LN = mybir.ActivationFunctionType.Ln

_CACHE = {}
DEBUG_DUMPS = False


def _build():
    nc = bacc.Bacc(None, target_bir_lowering=False)

    xT = nc.declare_dram_parameter("xT", [C, T], bf16, isOutput=False)
    wq = nc.declare_dram_parameter("wq", [128, 8, 512], bf16, isOutput=False)
    wk = nc.declare_dram_parameter("wk", [128, 8, 512], bf16, isOutput=False)
    wv = nc.declare_dram_parameter("wv", [128, 8, 512], bf16, isOutput=False)
    wp = nc.declare_dram_parameter("wp", [128, 4, 1024], bf16, isOutput=False)
    bq = nc.declare_dram_parameter("bq", [128, 4], f32, isOutput=False)
    bk = nc.declare_dram_parameter("bk", [128, 4], f32, isOutput=False)
    bv = nc.declare_dram_parameter("bv", [1, 512], bf16, isOutput=False)
    ones = nc.declare_dram_parameter("ones", [1, 128], bf16, isOutput=False)
    trimask = nc.declare_dram_parameter("trimask", [128, 128], bf16, isOutput=False)
    out = nc.declare_dram_parameter("out", [T, C], bf16, isOutput=True)
    if DEBUG_DUMPS:
        d_qt = nc.declare_dram_parameter("d_qt", [128, T], bf16, isOutput=True)
        d_kt = nc.declare_dram_parameter("d_kt", [128, T], bf16, isOutput=True)
        d_v = nc.declare_dram_parameter("d_v", [128, NPAIR, NST, 130], bf16, isOutput=True)
        d_yt = nc.declare_dram_parameter("d_yt", [128, NPAIR, T], bf16, isOutput=True)
        d_es = nc.declare_dram_parameter("d_es", [128, 512], bf16, isOutput=True)
        d_den = nc.declare_dram_parameter("d_den", [1, 1536], f32, isOutput=True)

    with tile.TileContext(nc) as tc:
        with tc.tile_pool(name="persist", bufs=1) as pp:
            # ---- persistent SBUF tensors ----
            xall = pp.tile([128, 8, T], bf16, tag="xall")
            wq_sb = pp.tile([128, 8, 512], bf16, tag="wq_sb")
            wk_sb = pp.tile([128, 8, 512], bf16, tag="wk_sb")
            wv_sb = pp.tile([128, 8, 512], bf16, tag="wv_sb")
            wp_sb = pp.tile([128, 4, 1024], bf16, tag="wp_sb")
            bq_sb = pp.tile([128, 4], f32, tag="bq")
            bk_sb = pp.tile([128, 4], f32, tag="bk")
            bv_sb = pp.tile([1, 512], bf16, tag="bv")
            ones_sb = pp.tile([1, 128], bf16, tag="ones")
            tri_sb = pp.tile([128, 128], bf16, tag="tri")
            bvrep = pp.tile([128, 512], f32, tag="bvrep")
            V = pp.tile([128, NPAIR, NST, 130], bf16, tag="V")
            YT = pp.tile([128, NPAIR, T], bf16, tag="YT")

            # x first (everything contracts over all of C), split for DMA-queue
            # parallelism; weights follow.
            for kk in range(8):
                for h in range(2):
                    nc.sync.dma_start(
                        xall[:, kk, 1024 * h : 1024 * h + 1024],
                        xT[128 * kk : 128 * kk + 128, 1024 * h : 1024 * h + 1024],
                    )
            nc.sync.dma_start(wq_sb[:], wq[:])
            nc.sync.dma_start(wk_sb[:], wk[:])
            nc.sync.dma_start(wv_sb[:], wv[:])
            nc.sync.dma_start(wp_sb[:], wp[:])
            nc.sync.dma_start(bq_sb[:], bq[:])
            nc.sync.dma_start(bk_sb[:], bk[:])
            nc.sync.dma_start(bv_sb[:], bv[:])
            nc.sync.dma_start(ones_sb[:], ones[:])
            nc.sync.dma_start(tri_sb[:], trimask[:])

            if DEBUG_DUMPS:
                es_dbg = pp.tile([128, 512], bf16, tag="es_dbg")
                den_dbg = pp.tile([1, 1536], f32, tag="den_dbg")

            onescol = pp.tile([128, 32], bf16, tag="onescol")
            nc.vector.memset(onescol[:], 1.0)
            for p in range(NPAIR):
                nc.vector.tensor_copy(
                    V[:, p, :, 64:130:65],
                    onescol[:, 0:32].rearrange("s (a b) -> s a b", a=16, b=2),
                )

            # PE emission-order chain, block granular
            _chain = {"prev": None, "first": None}

            def pe_mm(*args, **kw):
                inst = nc.tensor.matmul(*args, **kw)
                if _chain["first"] is None and _chain["prev"] is not None:
                    tile.add_dep_helper(
                        inst.ins, _chain["prev"].ins, sync=False,
                        reason="pe block order",
                    )
                if _chain["first"] is None:
                    _chain["first"] = inst
                _chain["prev"] = inst
                return inst

            def end_blk():
                _chain["first"] = None

            with (
                tc.tile_pool(name="qkt", bufs=2) as pqkt,
                tc.tile_pool(name="es", bufs=14) as pes,
                tc.tile_pool(name="rep", bufs=4) as prep,
                tc.tile_pool(name="ob", bufs=4) as pob,
                tc.tile_pool(name="pss", bufs=4, space="PSUM") as pss,
                tc.tile_pool(name="psy", bufs=2, space="PSUM") as psy,
            ):
                # bvrep = broadcast of bv to 128 partitions (K=1 matmul)
                psb = pss.tile([128, 512], f32, tag="ps", name="ps_bv")
                pe_mm(psb[:], ones_sb[:], bv_sb[:], start=True, stop=True)
                end_blk()
                nc.vector.tensor_copy(bvrep[:], psb[:])

                qt_of = {}
                kt_of = {}

                def alloc_qkt(p):
                    qt_of[p] = pqkt.tile([128, T], bf16, tag="QTp", name="QTp")
                    kt_of[p] = pqkt.tile([128, T], bf16, tag="KTp", name="KTp")

                # ---------- emission units ----------
                def v_unit(st):
                    def go():
                        ps = pss.tile([128, 512], f32, tag="ps", name="ps_v")
                        for kk in range(8):
                            pe_mm(
                                ps[:],
                                xall[:, kk, 128 * st : 128 * st + 128],
                                wv_sb[:, kk, :],
                                start=(kk == 0),
                                stop=(kk == 7),
                            )
                        end_blk()
                        srcv = ps.rearrange("s (p two d) -> s p two d", p=4, two=2)
                        bsrcv = bvrep.rearrange("s (p two d) -> s p two d", p=4, two=2)
                        for hh in range(2):
                            nc.vector.tensor_add(
                                V[:, :, st, 65 * hh : 65 * hh + 64],
                                srcv[:, :, hh, :],
                                bsrcv[:, :, hh, :],
                            )
                    return go

                def qk_unit(p, proj, tch):
                    def go():
                        w_sl = wq_sb if proj == "q" else wk_sb
                        dest = qt_of[p] if proj == "q" else kt_of[p]
                        bias_sb = bq_sb if proj == "q" else bk_sb
                        ps = pss.tile([128, 512], f32, tag="ps", name="ps_qk")
                        for kk in range(8):
                            pe_mm(
                                ps[:],
                                w_sl[:, kk, 128 * p : 128 * p + 128],
                                xall[:, kk, 512 * tch : 512 * tch + 512],
                                start=(kk == 0),
                                stop=(kk == 7),
                            )
                        end_blk()
                        nc.vector.tensor_scalar_add(
                            dest[:, 512 * tch : 512 * tch + 512],
                            ps[:],
                            bias_sb[:, p : p + 1],
                        )
                    return go

                def c_unit(m, e):
                    def go():
                        ps = pss.tile([128, 512], f32, tag="ps", name="ps_c")
                        for p4 in range(NPAIR):
                            pe_mm(
                                ps[:],
                                YT[:, p4, 128 * m : 128 * m + 128],
                                wp_sb[:, p4, 512 * e : 512 * e + 512],
                                start=(p4 == 0),
                                stop=(p4 == 3),
                            )
                        end_blk()
                        ob = pob.tile([128, 512], bf16, tag="ob", name="ob")
                        nc.vector.tensor_copy(ob[:], ps[:])
                        nc.sync.dma_start(
                            out[128 * m : 128 * m + 128, 512 * e : 512 * e + 512],
                            ob[:],
                        )
                    return go

                def emit_tail(p, j, psY):
                    for hh in range(2):
                        # custom-DVE recip reads SBUF only (PSUM-in misbehaves
                        # on hw): stage the den row through SBUF first.
                        denrow = prep.tile([1, 512], f32, tag="denrow", name="denrow")
                        nc.vector.tensor_copy(denrow[:], psY[hh][64:65, :])
                        row = prep.tile([1, 512], f32, tag="row", name="row")
                        nc.vector.reciprocal_approx_fast(row[:], denrow[:])
                        repc = prep.tile([64, 512], f32, tag="repc", name="repc")
                        nc.gpsimd.partition_broadcast(repc[:], row[:])
                        if DEBUG_DUMPS and p == 0 and j == 0 and hh == 0:
                            nc.vector.tensor_copy(den_dbg[:, 0:512], psY[hh][64:65, :])
                            nc.vector.tensor_copy(den_dbg[:, 512:1024], row[:])
                            nc.vector.tensor_copy(den_dbg[:, 1024:1536], repc[0:1, :])
                        nc.vector.tensor_mul(
                            YT[64 * hh : 64 * hh + 64, p, 512 * j : 512 * j + 512],
                            psY[hh][0:64, :],
                            repc[:],
                        )

                if DEBUG_DUMPS:
                    nc.sync.dma_start(d_qt[:], qt_of[0][:])
                    nc.sync.dma_start(d_kt[:], kt_of[0][:])

                # ---------- attention stages ----------
                GI = 2  # i-steps per attention block

                for stage in range(NPAIR):
                    p = stage

                    # filler list: own-pair QK first (gated incrementally by
                    # the S blocks), then V units (stage 0) / C units (stage 3)
                    filler = []
                    cgate = []   # stage-3: t-chunk whose tails a C unit needs
                    alloc_qkt(p)
                    qk_idx_of_tch = {}
                    for tch in range(NCH):
                        qk_idx_of_tch[tch] = len(filler)  # index of the q unit
                        for proj in ("q", "k"):
                            filler.append(qk_unit(p, proj, tch))
                            cgate.append(None)
                    n_qk = len(filler)
                    if stage == 0:
                        v_idx0 = len(filler)
                        filler += [v_unit(st) for st in range(NST)]
                        cgate += [None] * NST
                    if stage == NPAIR - 1:
                        for m in range(NST):
                            for e in range(2):
                                filler.append(c_unit(m, e))
                                cgate.append(m // 4)

                    def ensure_fill(upto):
                        nonlocal fidx
                        while fidx <= upto:
                            filler[fidx]()
                            fidx += 1

                    # stage 3 runs t-chunks descending so the last C units
                    # (gated on chunk-3 tails) unblock early
                    jorder = (
                        list(range(NCH - 1, -1, -1))
                        if stage == NPAIR - 1
                        else list(range(NCH))
                    )
                    blocks = []  # (j, [(i, hh)...], last_of_chunk)
                    for j in jorder:
                        nst_j = 4 * j + 4
                        for i0 in range(0, nst_j, GI):
                            ii = list(range(i0, min(i0 + GI, nst_j)))
                            steps = [(i, hh) for i in ii for hh in range(2)]
                            blocks.append((j, steps, i0 + GI >= nst_j))

                    nfill = len(filler)
                    nblk = len(blocks)

                    eS_store = {}
                    psY_of = {}
                    tails_pending = []
                    tails_done = set()
                    fidx = 0
                    for n in range(nblk + 3):
                        # deferred tails (release psY before next chunk's
                        # first AV allocates its slot)
                        while tails_pending and tails_pending[0][0] <= n:
                            _, tp, tj, tpsY = tails_pending.pop(0)
                            emit_tail(tp, tj, tpsY)
                            tails_done.add(tj)
                        # AV block n-3 (deeper S->AV lookahead)
                        if n >= 3:
                            j, steps, last = blocks[n - 3]
                            if stage == 0:
                                ensure_fill(
                                    v_idx0 + max(i for (i, _) in steps)
                                )
                            psYl = psY_of[j]
                            nst_j = 4 * j + 4
                            for (i, hh) in steps:
                                off = max(0, 128 * i - 512 * j)
                                eS = eS_store.pop((j, i, hh))
                                pe_mm(
                                    psYl[hh][:, off:512],
                                    V[:, p, i, 65 * hh : 65 * hh + 65],
                                    eS[:, off:512],
                                    start=(i == 0),
                                    stop=(i == nst_j - 1),
                                )
                            end_blk()
                            if last:
                                tails_pending.append((n + 1, p, j, psYl))
                        # filler unit(s), front-loaded
                        want = min(nfill, ((n + 1) * nfill) // max(1, int(0.7 * nblk)))
                        while fidx < want and (
                            cgate[fidx] is None or cgate[fidx] in tails_done
                        ):
                            filler[fidx]()
                            fidx += 1
                        # S block n
                        if n < nblk:
                            j, steps, last = blocks[n]
                            # own-pair QK for this t-chunk must be in place
                            ensure_fill(qk_idx_of_tch[j] + 1)
                            if j not in psY_of:
                                psY_of[j] = [
                                    psy.tile(
                                        [65, 512], f32,
                                        tag=f"psY{hh}", name=f"psY{hh}",
                                    )
                                    for hh in range(2)
                                ]
                            for (i, hh) in steps:
                                off = max(0, 128 * i - 512 * j)
                                h0 = 64 * hh
                                psS = pss.tile([128, 512], f32, tag="ps", name="psS")
                                pe_mm(
                                    psS[:, off:512],
                                    kt_of[p][h0 : h0 + 64, 128 * i : 128 * i + 128],
                                    qt_of[p][
                                        h0 : h0 + 64,
                                        512 * j + off : 512 * j + 512,
                                    ],
                                    start=True,
                                    stop=True,
                                )
                                eS = pes.tile([128, 512], bf16, tag="eS", name="eS")
                                nc.scalar.activation(
                                    eS[:, off:512], psS[:, off:512], EXP,
                                    scale=0.125,
                                )
                                if i >= 4 * j:
                                    nc.vector.tensor_mul(
                                        eS[:, off : off + 128],
                                        eS[:, off : off + 128],
                                        tri_sb[:],
                                    )
                                if (
                                    DEBUG_DUMPS and stage == 0 and j == 0
                                    and i == 0 and hh == 0
                                ):
                                    nc.vector.tensor_copy(es_dbg[:], eS[:])
                                eS_store[(j, i, hh)] = eS
                            end_blk()
                    # flush leftover fillers / tails
                    while tails_pending:
                        _, tp, tj, tpsY = tails_pending.pop(0)
                        emit_tail(tp, tj, tpsY)
                        tails_done.add(tj)
                    while fidx < nfill:
                        filler[fidx]()  # all tails emitted; gates satisfied
                        fidx += 1

                if DEBUG_DUMPS:
                    nc.sync.dma_start(d_v[:], V[:])
                    nc.sync.dma_start(d_yt[:], YT[:])
                    nc.sync.dma_start(d_es[:], es_dbg[:])
                    nc.sync.dma_start(d_den[:], den_dbg[:])

    nc.compile()
    return nc


def _to_bf16(a):
    return np.ascontiguousarray(a.astype(ml_dtypes.bfloat16))


def _prep_core_inputs(x, Wq, bq, Wk, bk, Wv, bv, Wp, core):
    b, hg = core // 2, core % 2
    h0 = 8 * hg

    def wprep(W):
        A = W[h0 : h0 + 8]
        Bm = np.transpose(A, (2, 0, 1)).reshape(C, 512)
        return _to_bf16(Bm.reshape(8, 128, 512).transpose(1, 0, 2))

    def bprep(bias):
        return np.ascontiguousarray(bias[h0 : h0 + 8].reshape(4, 128).T)

    wp_sl = Wp[:, 512 * hg : 512 * hg + 512]
    wp_prep = _to_bf16(wp_sl.T.reshape(4, 128, 1024).transpose(1, 0, 2))

    return {
        "xT": _to_bf16(x[b].T),
        "wq": wprep(Wq),
        "wk": wprep(Wk),
        "wv": wprep(Wv),
        "wp": wp_prep,
        "bq": bprep(bq),
        "bk": bprep(bk),
        "bv": _to_bf16(bv[h0 : h0 + 8].reshape(1, 512)),
        "ones": np.ones((1, 128), dtype=ml_dtypes.bfloat16),
        "trimask": _to_bf16(np.triu(np.ones((128, 128), np.float32))),
    }


TRACE = False
TRACE_KW = {}


def kernel(x, Wq, bq, Wk, bk, Wv, bv, Wp, bp):
    x = np.asarray(x, np.float32)
    Wq = np.asarray(Wq, np.float32)
    bq = np.asarray(bq, np.float32)
    Wk = np.asarray(Wk, np.float32)
    bk = np.asarray(bk, np.float32)
    Wv = np.asarray(Wv, np.float32)
    bv = np.asarray(bv, np.float32)
    Wp = np.asarray(Wp, np.float32)
    bp = np.asarray(bp, np.float32)

    if "nc" not in _CACHE:
        _CACHE["nc"] = _build()
    nc = _CACHE["nc"]

    in_maps = [
        _prep_core_inputs(x, Wq, bq, Wk, bk, Wv, bv, Wp, core)
        for core in range(NCORE)
    ]
    res = bass_utils.run_bass_kernel_spmd(
        nc, in_maps, list(range(NCORE)), trace=TRACE, **TRACE_KW
    )
    _CACHE["last_result"] = res

    outp = np.empty((B, T, C), np.float32)
    for b in range(B):
        outp[b] = res.results[2 * b]["out"] + res.results[2 * b + 1]["out"] + bp
    return outp
